# revision 1
# baseline (speedup 1.0000x reference)
"""Multi-head attention (B=4, T=2048, C=1024, H=16) on 8 TRN2 NeuronCores.

Self-contained grading entry point: kernel(**inputs) -> np.ndarray.

Sharding (batch x head-group tensor parallel): core c handles batch
b = c // 2 and head-group hg = c % 2 (8 of 16 heads = 512 of the 1024
model dims). Each core computes its 8 heads' attention for its batch
plus the partial output projection through its Wo row-slice; the host
sums the two head-group partials per batch and adds the output bias.

Per-core dataflow (Bass/Tile, single NeuronCore):
  phase 1: QT/KT [d, t] and V [s, d] projections from x^T (host-transposed)
  phase 2: per head-pair, per 512-wide t chunk: scoresT = K^T-layout
           row-paired matmuls -> exp on ACT -> context accumulation with a
           ones-row appended to V so the softmax denominator falls out of
           the same matmul; normalize via gpsimd partition-broadcast of the
           reciprocal row.
  phase 3: out[t, :] = ctxT.T @ Wo_slice, overlapped per t half.

Matmul operands are float32r (full-rate PE, ~tf32 multiply precision,
fp32 PSUM accumulation). Measured relative L2 error vs the fp32
reference: ~1.5e-4.
"""

import numpy as np

import concourse.bass as bass
import concourse.mybir as mybir
from concourse import bacc
from concourse.tile import TileContext
from concourse.bass_utils import run_bass_kernel_spmd

F32 = mybir.dt.float32
F32R = mybir.dt.float32r

T = 2048          # sequence length per core (one batch)
C = 1024          # model dim
D = 512           # head-group width (8 heads x 64)
DK = 64           # head dim
KC = 8            # k chunks of 128 over C
NP = 4            # head pairs per core
NG = 2            # t groups (halves of 1024)
TG = 1024         # t per group
SC = 16           # s chunks of 128 over T

_CACHE = {}


def _build_program(mm_dtype=F32R, n_cores=8, iters=1):
    nc = bacc.Bacc("TRN2", target_bir_lowering=False, debug=False,
                   num_devices=n_cores)

    xT = nc.dram_tensor("xT", [C, T], F32, kind="ExternalInput")
    wq = nc.dram_tensor("wq", [C, D], F32, kind="ExternalInput")
    wk = nc.dram_tensor("wk", [C, D], F32, kind="ExternalInput")
    wv = nc.dram_tensor("wv", [C, D], F32, kind="ExternalInput")
    wo = nc.dram_tensor("wo", [D, C], F32, kind="ExternalInput")
    bq = nc.dram_tensor("bq", [D], F32, kind="ExternalInput")
    bk = nc.dram_tensor("bk", [D], F32, kind="ExternalInput")
    bv = nc.dram_tensor("bv", [D], F32, kind="ExternalInput")
    out = nc.dram_tensor("out", [T, C], F32, kind="ExternalOutput")

    with TileContext(nc) as tc:
        with nc.allow_low_precision(reason="f32r matmul operands; f32 accum"):
            for _ in range(iters):
                _emit_body(nc, tc, xT, wq, wk, wv, wo, bq, bk, bv, out,
                           mm_dtype)

    nc.compile()
    return nc


def _emit_body(nc, tc, xT, wq, wk, wv, wo, bq, bk, bv, out, MD):
    EXP = mybir.ActivationFunctionType.Exp
    scale = 1.0 / np.sqrt(DK)

    def dma_cast(ap):
        # same-width bitcast so fp32 DRAM loads into mm_dtype tiles
        return ap.bitcast(MD) if MD != F32 else ap

    with tc.tile_pool(name="persist", bufs=1) as pp:
        # --- constants ---
        bq_sb = pp.tile([128, 4], F32, tag="bq_sb")
        nc.sync.dma_start(bq_sb[:], bq.ap().rearrange("(j p) -> p j", p=128))
        bk_sb = pp.tile([128, 4], F32, tag="bk_sb")
        nc.sync.dma_start(bk_sb[:], bk.ap().rearrange("(j p) -> p j", p=128))
        bv_row = pp.tile([1, D], MD, tag="bv_row")
        nc.sync.dma_start(bv_row[0:1, :], dma_cast(bv.ap().unsqueeze(0)))
        ones_f = pp.tile([128, 128], F32, tag="ones_f")
        nc.vector.memset(ones_f[:], 1.0)
        ones_sb = pp.tile([128, 128], MD, tag="ones_sb")
        nc.vector.tensor_copy(ones_sb[:], ones_f[:])

        # qkv activations (live through phases 1-2)
        qt_sb = [pp.tile([128, T], MD, tag=f"qt{j}", name=f"qt{j}")
                 for j in range(4)]
        kt_sb = [pp.tile([128, T], MD, tag=f"kt{j}", name=f"kt{j}")
                 for j in range(4)]
        # V with per-head ones column: head h occupies cols h*65..h*65+63,
        # ones at h*65+64 -> [128, 520]
        v_sb = [pp.tile([128, 8 * 65], MD, tag=f"v{si}", name=f"v{si}")
                for si in range(SC)]

        # broadcast bv across partitions via K=1 matmul
        with tc.tile_pool(name="ph0ps", bufs=1, space="PSUM") as ps0:
            bvb_ps = ps0.tile([128, D], F32, tag="bvb_ps")
            nc.tensor.matmul(bvb_ps[:], ones_sb[0:1, :], bv_row[0:1, :],
                             start=True, stop=True)
            bvb = pp.tile([128, D], F32, tag="bvb")
            nc.vector.tensor_copy(bvb[:], bvb_ps[:])

        # ---------------- phase 1: projections ----------------
        with tc.tile_pool(name="wpool", bufs=1) as wp, \
             tc.tile_pool(name="xpool", bufs=16) as xp, \
             tc.tile_pool(name="pj_ps", bufs=8, space="PSUM") as pjp:
            # x quarter 0 first so the first matmuls aren't queued behind 6MB
            # of weights on the HWDGE queue; then weights per-k-chunk.
            def load_x_quarter(q):
                t0 = q * 512
                xt = []
                for kk in range(KC):
                    xtile = xp.tile([128, 512], MD, tag="xt", name=f"xt{q}_{kk}")
                    nc.sync.dma_start(
                        xtile[:],
                        dma_cast(xT[kk * 128:(kk + 1) * 128, t0:t0 + 512]))
                    xt.append(xtile)
                return xt

            xt0 = load_x_quarter(0)
            wq_sb = wp.tile([128, KC, D], MD, tag="wq_sb")
            wk_sb = wp.tile([128, KC, D], MD, tag="wk_sb")
            wv_sb = wp.tile([128, KC, D], MD, tag="wv_sb")
            for t_sb, t_dr in ((wq_sb, wq), (wk_sb, wk), (wv_sb, wv)):
                for kk in range(KC):
                    nc.sync.dma_start(
                        t_sb[:, kk, :],
                        dma_cast(t_dr[kk * 128:(kk + 1) * 128, :]))

            for q in range(4):        # t quarters of 512
                t0 = q * 512
                xt = xt0 if q == 0 else load_x_quarter(q)
                # QT[d, t] += wq[k, d].T @ xT[k, t]
                for j in range(4):
                    ps = pjp.tile([128, 512], F32, tag="pj", name=f"pq{q}_{j}")
                    for kk in range(KC):
                        nc.tensor.matmul(
                            ps[:], wq_sb[:, kk, j * 128:(j + 1) * 128],
                            xt[kk][:],
                            start=(kk == 0), stop=(kk == KC - 1))
                    nc.vector.tensor_scalar_add(
                        qt_sb[j][:, t0:t0 + 512], ps[:], bq_sb[:, j:j + 1])
                for j in range(4):
                    ps = pjp.tile([128, 512], F32, tag="pj", name=f"pk{q}_{j}")
                    for kk in range(KC):
                        nc.tensor.matmul(
                            ps[:], wk_sb[:, kk, j * 128:(j + 1) * 128],
                            xt[kk][:],
                            start=(kk == 0), stop=(kk == KC - 1))
                    nc.vector.tensor_scalar_add(
                        kt_sb[j][:, t0:t0 + 512], ps[:], bk_sb[:, j:j + 1])
                # V[s, d] += xT[k, s].T @ wv[k, d]
                for sl in range(4):
                    si = q * 4 + sl
                    ps = pjp.tile([128, 512], F32, tag="pj", name=f"pv{q}_{sl}")
                    for kk in range(KC):
                        nc.tensor.matmul(
                            ps[:], xt[kk][:, sl * 128:(sl + 1) * 128],
                            wv_sb[:, kk, :],
                            start=(kk == 0), stop=(kk == KC - 1))
                    vdst = v_sb[si].rearrange("p (h e) -> p h e", e=65)
                    nc.vector.tensor_add(
                        vdst[:, :, 0:64],
                        ps.rearrange("p (h e) -> p h e", e=64),
                        bvb.rearrange("p (h e) -> p h e", e=64))
                    nc.vector.tensor_copy(vdst[:, :, 64:65],
                                          ones_f[:, 0:8].unsqueeze(2))

        # ---------------- phases 2+3: attention + output proj ----------------
        with tc.tile_pool(name="attn", bufs=1) as ap_, \
             tc.tile_pool(name="exp_pool", bufs=4) as ep, \
             tc.tile_pool(name="sc_ps", bufs=2, space="PSUM") as scp, \
             tc.tile_pool(name="cx_ps", bufs=4, space="PSUM") as cxp, \
             tc.tile_pool(name="out_pool", bufs=2) as op_:

            wo_sb = ap_.tile([128, 4, C], MD, tag="wo_sb")
            nc.sync.dma_start(
                wo_sb[:], dma_cast(wo.ap().rearrange("(a p) c -> p a c", p=128)))
            ctx_sb = [ap_.tile([128, T], MD, tag=f"ctx{p}", name=f"ctx{p}")
                      for p in range(NP)]

            for g in range(NG):       # t halves of 1024
                gt = g * TG
                for p in range(NP):   # head pairs
                    hA, hB = 2 * p, 2 * p + 1
                    for tt in range(2):
                        q0 = gt + tt * 512
                        # ctx accumulators: [65, 512] per head
                        cxA = cxp.tile([128, 512], F32, tag="cx",
                                       name=f"cxA{g}{p}{tt}")
                        cxB = cxp.tile([128, 512], F32, tag="cx",
                                       name=f"cxB{g}{p}{tt}")
                        for si in range(SC):
                            s0 = si * 128
                            # scoresT psum: [A | B] cols of 512; the two
                            # K=64 matmuls land in row-groups 0-63/64-127
                            # and stream concurrently on the PE
                            sc = scp.tile([128, 1024], F32, tag="sc",
                                          name=f"sc{g}{p}{tt}{si}")
                            nc.tensor.matmul(
                                sc[:, 0:512],
                                kt_sb[p][0:64, s0:s0 + 128],
                                qt_sb[p][0:64, q0:q0 + 512],
                                start=True, stop=True)
                            nc.tensor.matmul(
                                sc[:, 512:1024],
                                kt_sb[p][64:128, s0:s0 + 128],
                                qt_sb[p][64:128, q0:q0 + 512],
                                start=True, stop=True)
                            et = ep.tile([128, 1024], MD, tag="et",
                                         name=f"et{g}{p}{tt}{si}")
                            nc.scalar.activation(et[:], sc[:], EXP, scale=scale)
                            # ctx[d(+denom), t] += Vaug[s, d].T @ expT[s, t]
                            nc.tensor.matmul(
                                cxA[0:65, :],
                                v_sb[si][:, hA * 65:hA * 65 + 65],
                                et[:, 0:512],
                                start=(si == 0), stop=(si == SC - 1))
                            nc.tensor.matmul(
                                cxB[0:65, :],
                                v_sb[si][:, hB * 65:hB * 65 + 65],
                                et[:, 512:1024],
                                start=(si == 0), stop=(si == SC - 1))
                        # normalize: ctxT[d, t] = ctx[d, t] * (1/denom[t])
                        for cxi, h in ((cxA, hA), (cxB, hB)):
                            rc = ep.tile([1, 512], F32, tag="rc", bufs=4,
                                         name=f"rc{g}{p}{tt}{h % 2}")
                            nc.vector.reciprocal(rc[0:1, :], cxi[64:65, :])
                            o = (h % 2) * 64
                            bc = ep.tile([128, 512], F32, tag="bcs", bufs=4,
                                         name=f"bcs{g}{p}{tt}{h % 2}")
                            nc.gpsimd.partition_broadcast(bc[:, :], rc[0:1, :],
                                                          channels=128)
                            dst = ctx_sb[p][o:o + 64, q0:q0 + 512]
                            nc.vector.tensor_copy(dst, cxi[0:64, :])
                            nc.vector.tensor_mul(dst, dst, bc[o:o + 64, :])

                # output projection for this t half
                for t8 in range(8):
                    tb = gt + t8 * 128
                    ob = op_.tile([128, C], F32, tag="ob", name=f"ob{g}{t8}")
                    for cc in range(2):
                        po = cxp.tile([128, 512], F32, tag="cx",
                                      name=f"po{g}{t8}{cc}")
                        for dd in range(4):
                            nc.tensor.matmul(
                                po[:], ctx_sb[dd][:, tb:tb + 128],
                                wo_sb[:, dd, cc * 512:(cc + 1) * 512],
                                start=(dd == 0), stop=(dd == 3))
                        nc.vector.tensor_copy(ob[:, cc * 512:(cc + 1) * 512],
                                              po[:])
                    nc.sync.dma_start(out[tb:tb + 128, :], ob[:])


def _shard_inputs(x, Wq, bq, Wk, bk, Wv, bv, Wo):
    in_maps = []
    for c in range(8):
        b, hg = c // 2, c % 2
        sl = slice(hg * D, (hg + 1) * D)
        in_maps.append({
            "xT": np.ascontiguousarray(np.asarray(x[b]).T),
            "wq": np.ascontiguousarray(np.asarray(Wq[:, sl])),
            "wk": np.ascontiguousarray(np.asarray(Wk[:, sl])),
            "wv": np.ascontiguousarray(np.asarray(Wv[:, sl])),
            "wo": np.ascontiguousarray(np.asarray(Wo[sl, :])),
            "bq": np.ascontiguousarray(np.asarray(bq[sl])),
            "bk": np.ascontiguousarray(np.asarray(bk[sl])),
            "bv": np.ascontiguousarray(np.asarray(bv[sl])),
        })
    return in_maps


def kernel(x, Wq, bq, Wk, bk, Wv, bv, Wo, bo):
    """Full inputs in, full [4, 2048, 1024] float32 output out."""
    if "nc" not in _CACHE:
        _CACHE["nc"] = _build_program()
    nc = _CACHE["nc"]

    in_maps = _shard_inputs(x, Wq, bq, Wk, bk, Wv, bv, Wo)
    res = run_bass_kernel_spmd(nc, in_maps, list(range(8)))

    bo = np.asarray(bo, dtype=np.float32)
    out = np.empty((4, T, C), np.float32)
    for b in range(4):
        out[b] = res.results[2 * b]["out"] + res.results[2 * b + 1]["out"] + bo
    return out


# revision 3
# speedup vs baseline: 1.0185x; 1.0185x over previous
"""Multi-head attention (B=4, T=2048, C=1024, H=16) on 8 TRN2 NeuronCores.

Self-contained grading entry point: kernel(**inputs) -> np.ndarray.

Sharding (batch x head-group tensor parallel): core c handles batch
b = c // 2 and head-group hg = c % 2 (8 of 16 heads = 512 of the 1024
model dims). Each core computes its 8 heads' attention for its batch
plus the partial output projection through its Wo row-slice; the host
sums the two head-group partials per batch and adds the output bias.

Per-core dataflow (Bass/Tile, single NeuronCore):
  phase 1: QT/KT [d, t] and V [s, d] projections from x^T (host-transposed)
  phase 2: per head-pair, per 512-wide t chunk: scoresT = K^T-layout
           row-paired matmuls -> exp on ACT -> context accumulation with a
           ones-row appended to V so the softmax denominator falls out of
           the same matmul; normalize via gpsimd partition-broadcast of the
           reciprocal row.
  phase 3: out[t, :] = ctxT.T @ Wo_slice, overlapped per t half.

Matmul operands are float32r (full-rate PE, ~tf32 multiply precision,
fp32 PSUM accumulation). Measured relative L2 error vs the fp32
reference: ~1.5e-4.
"""

import numpy as np

import concourse.bass as bass
import concourse.mybir as mybir
from concourse import bacc
from concourse.tile import TileContext

F32 = mybir.dt.float32
F32R = mybir.dt.float32r

T = 2048          # sequence length per core (one batch)
C = 1024          # model dim
D = 512           # head-group width (8 heads x 64)
DK = 64           # head dim
KC = 8            # k chunks of 128 over C
NP = 4            # head pairs per core
NG = 2            # t groups (halves of 1024)
TG = 1024         # t per group
SC = 16           # s chunks of 128 over T

_CACHE = {}


def _build_program(mm_dtype=F32R, n_cores=8, iters=1):
    nc = bacc.Bacc("TRN2", target_bir_lowering=False, debug=False,
                   num_devices=n_cores)

    xT = nc.dram_tensor("xT", [C, T], F32, kind="ExternalInput")
    wq = nc.dram_tensor("wq", [C, D], F32, kind="ExternalInput")
    wk = nc.dram_tensor("wk", [C, D], F32, kind="ExternalInput")
    wv = nc.dram_tensor("wv", [C, D], F32, kind="ExternalInput")
    wo = nc.dram_tensor("wo", [D, C], F32, kind="ExternalInput")
    bq = nc.dram_tensor("bq", [D], F32, kind="ExternalInput")
    bk = nc.dram_tensor("bk", [D], F32, kind="ExternalInput")
    bv = nc.dram_tensor("bv", [D], F32, kind="ExternalInput")
    out = nc.dram_tensor("out", [T, C], F32, kind="ExternalOutput")

    with TileContext(nc) as tc:
        with nc.allow_low_precision(reason="f32r matmul operands; f32 accum"):
            for _ in range(iters):
                _emit_body(nc, tc, xT, wq, wk, wv, wo, bq, bk, bv, out,
                           mm_dtype)

    nc.compile()
    return nc


def _emit_body(nc, tc, xT, wq, wk, wv, wo, bq, bk, bv, out, MD):
    EXP = mybir.ActivationFunctionType.Exp
    scale = 1.0 / np.sqrt(DK)

    def dma_cast(ap):
        # same-width bitcast so fp32 DRAM loads into mm_dtype tiles
        return ap.bitcast(MD) if MD != F32 else ap

    with tc.tile_pool(name="persist", bufs=1) as pp:
        # --- constants ---
        bq_sb = pp.tile([128, 4], F32, tag="bq_sb")
        nc.sync.dma_start(bq_sb[:], bq.ap().rearrange("(j p) -> p j", p=128))
        bk_sb = pp.tile([128, 4], F32, tag="bk_sb")
        nc.sync.dma_start(bk_sb[:], bk.ap().rearrange("(j p) -> p j", p=128))
        bv_row = pp.tile([1, D], MD, tag="bv_row")
        nc.sync.dma_start(bv_row[0:1, :], dma_cast(bv.ap().unsqueeze(0)))
        ones_f = pp.tile([128, 128], F32, tag="ones_f")
        nc.vector.memset(ones_f[:], 1.0)
        ones_sb = pp.tile([128, 128], MD, tag="ones_sb")
        nc.vector.tensor_copy(ones_sb[:], ones_f[:])

        # qkv activations (live through phases 1-2)
        qt_sb = [pp.tile([128, T], MD, tag=f"qt{j}", name=f"qt{j}")
                 for j in range(4)]
        kt_sb = [pp.tile([128, T], MD, tag=f"kt{j}", name=f"kt{j}")
                 for j in range(4)]
        # V with per-head ones column: head h occupies cols h*65..h*65+63,
        # ones at h*65+64 -> [128, 520]
        v_sb = [pp.tile([128, 8 * 65], MD, tag=f"v{si}", name=f"v{si}")
                for si in range(SC)]

        # broadcast bv across partitions via K=1 matmul
        with tc.tile_pool(name="ph0ps", bufs=1, space="PSUM") as ps0:
            bvb_ps = ps0.tile([128, D], F32, tag="bvb_ps")
            nc.tensor.matmul(bvb_ps[:], ones_sb[0:1, :], bv_row[0:1, :],
                             start=True, stop=True)
            bvb = pp.tile([128, D], F32, tag="bvb")
            nc.vector.tensor_copy(bvb[:], bvb_ps[:])

        # ---------------- phase 1: projections ----------------
        with tc.tile_pool(name="wpool", bufs=1) as wp, \
             tc.tile_pool(name="xpool", bufs=16) as xp, \
             tc.tile_pool(name="pj_ps", bufs=8, space="PSUM") as pjp:
            # x quarter 0 first so the first matmuls aren't queued behind 6MB
            # of weights on the HWDGE queue; then weights per-k-chunk.
            def load_x_quarter(q):
                t0 = q * 512
                xt = []
                for kk in range(KC):
                    xtile = xp.tile([128, 512], MD, tag="xt", name=f"xt{q}_{kk}")
                    nc.sync.dma_start(
                        xtile[:],
                        dma_cast(xT[kk * 128:(kk + 1) * 128, t0:t0 + 512]))
                    xt.append(xtile)
                return xt

            xt0 = load_x_quarter(0)
            wq_sb = wp.tile([128, KC, D], MD, tag="wq_sb")
            wk_sb = wp.tile([128, KC, D], MD, tag="wk_sb")
            wv_sb = wp.tile([128, KC, D], MD, tag="wv_sb")
            for t_sb, t_dr in ((wq_sb, wq), (wk_sb, wk), (wv_sb, wv)):
                for kk in range(KC):
                    nc.sync.dma_start(
                        t_sb[:, kk, :],
                        dma_cast(t_dr[kk * 128:(kk + 1) * 128, :]))

            for q in range(4):        # t quarters of 512
                t0 = q * 512
                xt = xt0 if q == 0 else load_x_quarter(q)
                # QT[d, t] += wq[k, d].T @ xT[k, t]
                for j in range(4):
                    ps = pjp.tile([128, 512], F32, tag="pj", name=f"pq{q}_{j}")
                    for kk in range(KC):
                        nc.tensor.matmul(
                            ps[:], wq_sb[:, kk, j * 128:(j + 1) * 128],
                            xt[kk][:],
                            start=(kk == 0), stop=(kk == KC - 1))
                    nc.vector.tensor_scalar_add(
                        qt_sb[j][:, t0:t0 + 512], ps[:], bq_sb[:, j:j + 1])
                for j in range(4):
                    ps = pjp.tile([128, 512], F32, tag="pj", name=f"pk{q}_{j}")
                    for kk in range(KC):
                        nc.tensor.matmul(
                            ps[:], wk_sb[:, kk, j * 128:(j + 1) * 128],
                            xt[kk][:],
                            start=(kk == 0), stop=(kk == KC - 1))
                    nc.vector.tensor_scalar_add(
                        kt_sb[j][:, t0:t0 + 512], ps[:], bk_sb[:, j:j + 1])
                # V[s, d] += xT[k, s].T @ wv[k, d]
                for sl in range(4):
                    si = q * 4 + sl
                    ps = pjp.tile([128, 512], F32, tag="pj", name=f"pv{q}_{sl}")
                    for kk in range(KC):
                        nc.tensor.matmul(
                            ps[:], xt[kk][:, sl * 128:(sl + 1) * 128],
                            wv_sb[:, kk, :],
                            start=(kk == 0), stop=(kk == KC - 1))
                    vdst = v_sb[si].rearrange("p (h e) -> p h e", e=65)
                    nc.vector.tensor_add(
                        vdst[:, :, 0:64],
                        ps.rearrange("p (h e) -> p h e", e=64),
                        bvb.rearrange("p (h e) -> p h e", e=64))
                    nc.vector.tensor_copy(vdst[:, :, 64:65],
                                          ones_f[:, 0:8].unsqueeze(2))

        # ---------------- phases 2+3: attention + output proj ----------------
        with tc.tile_pool(name="attn", bufs=1) as ap_, \
             tc.tile_pool(name="exp_pool", bufs=4) as ep, \
             tc.tile_pool(name="sc_ps", bufs=2, space="PSUM") as scp, \
             tc.tile_pool(name="cx_ps", bufs=4, space="PSUM") as cxp, \
             tc.tile_pool(name="out_pool", bufs=2) as op_:

            wo_sb = ap_.tile([128, 4, C], MD, tag="wo_sb")
            nc.sync.dma_start(
                wo_sb[:], dma_cast(wo.ap().rearrange("(a p) c -> p a c", p=128)))
            ctx_sb = [ap_.tile([128, T], MD, tag=f"ctx{p}", name=f"ctx{p}")
                      for p in range(NP)]

            for g in range(NG):       # t halves of 1024
                gt = g * TG
                for p in range(NP):   # head pairs
                    hA, hB = 2 * p, 2 * p + 1
                    for tt in range(2):
                        q0 = gt + tt * 512
                        # ctx accumulators: [65, 512] per head
                        cxA = cxp.tile([128, 512], F32, tag="cx",
                                       name=f"cxA{g}{p}{tt}")
                        cxB = cxp.tile([128, 512], F32, tag="cx",
                                       name=f"cxB{g}{p}{tt}")
                        for si in range(SC):
                            s0 = si * 128
                            # scoresT psum: [A | B] cols of 512; the two
                            # K=64 matmuls land in row-groups 0-63/64-127
                            # and stream concurrently on the PE
                            sc = scp.tile([128, 1024], F32, tag="sc",
                                          name=f"sc{g}{p}{tt}{si}")
                            nc.tensor.matmul(
                                sc[:, 0:512],
                                kt_sb[p][0:64, s0:s0 + 128],
                                qt_sb[p][0:64, q0:q0 + 512],
                                start=True, stop=True)
                            nc.tensor.matmul(
                                sc[:, 512:1024],
                                kt_sb[p][64:128, s0:s0 + 128],
                                qt_sb[p][64:128, q0:q0 + 512],
                                start=True, stop=True)
                            et = ep.tile([128, 1024], MD, tag="et",
                                         name=f"et{g}{p}{tt}{si}")
                            nc.scalar.activation(et[:], sc[:], EXP, scale=scale)
                            # ctx[d(+denom), t] += Vaug[s, d].T @ expT[s, t]
                            nc.tensor.matmul(
                                cxA[0:65, :],
                                v_sb[si][:, hA * 65:hA * 65 + 65],
                                et[:, 0:512],
                                start=(si == 0), stop=(si == SC - 1))
                            nc.tensor.matmul(
                                cxB[0:65, :],
                                v_sb[si][:, hB * 65:hB * 65 + 65],
                                et[:, 512:1024],
                                start=(si == 0), stop=(si == SC - 1))
                        # normalize: ctxT[d, t] = ctx[d, t] * (1/denom[t])
                        for cxi, h in ((cxA, hA), (cxB, hB)):
                            rc = ep.tile([1, 512], F32, tag="rc", bufs=4,
                                         name=f"rc{g}{p}{tt}{h % 2}")
                            nc.vector.reciprocal(rc[0:1, :], cxi[64:65, :])
                            o = (h % 2) * 64
                            bc = ep.tile([128, 512], F32, tag="bcs", bufs=4,
                                         name=f"bcs{g}{p}{tt}{h % 2}")
                            nc.gpsimd.partition_broadcast(bc[:, :], rc[0:1, :],
                                                          channels=128)
                            dst = ctx_sb[p][o:o + 64, q0:q0 + 512]
                            nc.vector.tensor_copy(dst, cxi[0:64, :])
                            nc.vector.tensor_mul(dst, dst, bc[o:o + 64, :])

                # output projection for this t half
                for t8 in range(8):
                    tb = gt + t8 * 128
                    ob = op_.tile([128, C], F32, tag="ob", name=f"ob{g}{t8}")
                    for cc in range(2):
                        po = cxp.tile([128, 512], F32, tag="cx",
                                      name=f"po{g}{t8}{cc}")
                        for dd in range(4):
                            nc.tensor.matmul(
                                po[:], ctx_sb[dd][:, tb:tb + 128],
                                wo_sb[:, dd, cc * 512:(cc + 1) * 512],
                                start=(dd == 0), stop=(dd == 3))
                        nc.vector.tensor_copy(ob[:, cc * 512:(cc + 1) * 512],
                                              po[:])
                    nc.sync.dma_start(out[tb:tb + 128, :], ob[:])


def _shard_inputs(x, Wq, bq, Wk, bk, Wv, bv, Wo):
    in_maps = []
    for c in range(8):
        b, hg = c // 2, c % 2
        sl = slice(hg * D, (hg + 1) * D)
        in_maps.append({
            "xT": np.ascontiguousarray(np.asarray(x[b]).T),
            "wq": np.ascontiguousarray(np.asarray(Wq[:, sl])),
            "wk": np.ascontiguousarray(np.asarray(Wk[:, sl])),
            "wv": np.ascontiguousarray(np.asarray(Wv[:, sl])),
            "wo": np.ascontiguousarray(np.asarray(Wo[sl, :])),
            "bq": np.ascontiguousarray(np.asarray(bq[sl])),
            "bk": np.ascontiguousarray(np.asarray(bk[sl])),
            "bv": np.ascontiguousarray(np.asarray(bv[sl])),
        })
    return in_maps


class _Runner:
    """Persistent-jit SPMD runner (mirrors bass2jax.run_bass_via_pjrt but
    keeps the jitted callable alive so repeat kernel() calls don't
    re-trace). Our kernel writes every output element, so output buffers
    are passed non-donated."""

    def __init__(self, nc, n_cores=8):
        import jax
        from jax.sharding import Mesh, PartitionSpec
        from jax.experimental.shard_map import shard_map
        from concourse.bass2jax import (_bass_exec_p, install_neuronx_cc_hook,
                                        partition_id_tensor)
        install_neuronx_cc_hook()
        self.jax = jax
        self.n_cores = n_cores
        pid = nc.partition_id_tensor
        pid_name = pid.name if pid is not None else None

        in_names, out_names, out_avals, out_shapes = [], [], [], []
        for alloc in nc.m.functions[0].allocations:
            if not isinstance(alloc, mybir.MemoryLocationSet):
                continue
            name = alloc.memorylocations[0].name
            if alloc.kind == "ExternalInput":
                if name != pid_name:
                    in_names.append(name)
            elif alloc.kind == "ExternalOutput":
                out_names.append(name)
                shape = tuple(alloc.tensor_shape)
                out_shapes.append(shape)
                out_avals.append(
                    jax.core.ShapedArray(shape, mybir.dt.np(alloc.dtype)))
        self.in_names = in_names
        self.out_names = out_names
        self.out_shapes = out_shapes
        n_params = len(in_names)
        all_in_names = list(in_names) + list(out_names)
        if pid_name is not None:
            all_in_names.append(pid_name)

        def _body(*args):
            operands = list(args)
            if pid_name is not None:
                operands.append(partition_id_tensor())
            return tuple(_bass_exec_p.bind(
                *operands,
                out_avals=tuple(out_avals),
                in_names=tuple(all_in_names),
                out_names=tuple(out_names),
                lowering_input_output_aliases=(),
                sim_require_finite=True,
                sim_require_nnan=True,
                nc=nc,
            ))

        devices = jax.devices()[:n_cores]
        mesh = Mesh(np.asarray(devices), ("core",))
        n_outs = len(out_names)
        self.fn = jax.jit(shard_map(
            _body, mesh=mesh,
            in_specs=(PartitionSpec("core"),) * (n_params + n_outs),
            out_specs=(PartitionSpec("core"),) * n_outs,
            check_rep=False), keep_unused=True)
        self.zero_outs = [
            np.zeros((n_cores * s[0], *s[1:]), np.float32) for s in out_shapes]

    def run(self, in_maps):
        args = [np.concatenate([np.asarray(m[name]) for m in in_maps], axis=0)
                for name in self.in_names]
        out_arrs = self.fn(*args, *self.zero_outs)
        self.jax.block_until_ready(out_arrs)
        per_core = []
        for c in range(self.n_cores):
            d = {}
            for i, name in enumerate(self.out_names):
                shape = self.out_shapes[i]
                d[name] = np.asarray(out_arrs[i]).reshape(
                    self.n_cores, *shape)[c]
            per_core.append(d)
        return per_core


def kernel(x, Wq, bq, Wk, bk, Wv, bv, Wo, bo):
    """Full inputs in, full [4, 2048, 1024] float32 output out."""
    if "runner" not in _CACHE:
        _CACHE["runner"] = _Runner(_build_program())
    in_maps = _shard_inputs(x, Wq, bq, Wk, bk, Wv, bv, Wo)
    results = _CACHE["runner"].run(in_maps)

    bo = np.asarray(bo, dtype=np.float32)
    out = np.empty((4, T, C), np.float32)
    for b in range(4):
        out[b] = results[2 * b]["out"] + results[2 * b + 1]["out"] + bo
    return out


# revision 4
# speedup vs baseline: 1.1655x; 1.1442x over previous
"""Multi-head attention (B=4, T=2048, C=1024, H=16) on 8 TRN2 NeuronCores.

Self-contained grading entry point: kernel(**inputs) -> np.ndarray.

Sharding (batch x head-group tensor parallel): core c handles batch
b = c // 2 and head-group hg = c % 2 (8 of 16 heads = 512 of the 1024
model dims). Each core computes its 8 heads' attention for its batch
plus the partial output projection through its Wo row-slice; the host
sums the two head-group partials per batch and adds the output bias.

Per-core dataflow (Bass/Tile, single NeuronCore):
  phase 1: QT/KT [d, t] and V [s, d] projections from x^T (host-transposed)
  phase 2: per head-pair, per 512-wide t chunk: scoresT = K^T-layout
           row-paired matmuls -> exp on ACT -> context accumulation with a
           ones-row appended to V so the softmax denominator falls out of
           the same matmul; normalize via gpsimd partition-broadcast of the
           reciprocal row.
  phase 3: out[t, :] = ctxT.T @ Wo_slice, overlapped per t half.

Matmul operands are float32r (full-rate PE, ~tf32 multiply precision,
fp32 PSUM accumulation). Measured relative L2 error vs the fp32
reference: ~1.5e-4.
"""

import numpy as np

import concourse.bass as bass
import concourse.mybir as mybir
from concourse import bacc
from concourse.tile import TileContext

F32 = mybir.dt.float32
F32R = mybir.dt.float32r

T = 2048          # sequence length per core (one batch)
C = 1024          # model dim
D = 512           # head-group width (8 heads x 64)
DK = 64           # head dim
KC = 8            # k chunks of 128 over C
NP = 4            # head pairs per core
NG = 2            # t groups (halves of 1024)
TG = 1024         # t per group
SC = 16           # s chunks of 128 over T

_CACHE = {}


def _build_program(mm_dtype=F32R, n_cores=8, iters=1):
    nc = bacc.Bacc("TRN2", target_bir_lowering=False, debug=False,
                   num_devices=n_cores)

    xT = nc.dram_tensor("xT", [C, T], F32, kind="ExternalInput")
    wq = nc.dram_tensor("wq", [C, D], F32, kind="ExternalInput")
    wk = nc.dram_tensor("wk", [C, D], F32, kind="ExternalInput")
    wv = nc.dram_tensor("wv", [C, D], F32, kind="ExternalInput")
    wo = nc.dram_tensor("wo", [D, C], F32, kind="ExternalInput")
    bq = nc.dram_tensor("bq", [D], F32, kind="ExternalInput")
    bk = nc.dram_tensor("bk", [D], F32, kind="ExternalInput")
    bv = nc.dram_tensor("bv", [D], F32, kind="ExternalInput")
    out = nc.dram_tensor("out", [T, C], F32, kind="ExternalOutput")

    with TileContext(nc) as tc:
        with nc.allow_low_precision(reason="f32r matmul operands; f32 accum"):
            for _ in range(iters):
                _emit_body(nc, tc, xT, wq, wk, wv, wo, bq, bk, bv, out,
                           mm_dtype)

    nc.compile()
    return nc


def _emit_body(nc, tc, xT, wq, wk, wv, wo, bq, bk, bv, out, MD):
    EXP = mybir.ActivationFunctionType.Exp
    scale = 1.0 / np.sqrt(DK)

    def dma_cast(ap):
        # same-width bitcast so fp32 DRAM loads into mm_dtype tiles
        return ap.bitcast(MD) if MD != F32 else ap

    with tc.tile_pool(name="persist", bufs=1) as pp:
        # --- constants ---
        bq_sb = pp.tile([128, 4], F32, tag="bq_sb")
        nc.sync.dma_start(bq_sb[:], bq.ap().rearrange("(j p) -> p j", p=128))
        bk_sb = pp.tile([128, 4], F32, tag="bk_sb")
        nc.sync.dma_start(bk_sb[:], bk.ap().rearrange("(j p) -> p j", p=128))
        bv_row = pp.tile([1, D], MD, tag="bv_row")
        nc.sync.dma_start(bv_row[0:1, :], dma_cast(bv.ap().unsqueeze(0)))
        ones_f = pp.tile([128, 128], F32, tag="ones_f")
        nc.vector.memset(ones_f[:], 1.0)
        ones_sb = pp.tile([128, 128], MD, tag="ones_sb")
        nc.vector.tensor_copy(ones_sb[:], ones_f[:])

        # qkv activations (live through phases 1-2)
        qt_sb = [pp.tile([128, T], MD, tag=f"qt{j}", name=f"qt{j}")
                 for j in range(4)]
        kt_sb = [pp.tile([128, T], MD, tag=f"kt{j}", name=f"kt{j}")
                 for j in range(4)]
        # V with per-head ones column: head h occupies cols h*65..h*65+63,
        # ones at h*65+64 -> [128, 520]
        v_sb = [pp.tile([128, 8 * 65], MD, tag=f"v{si}", name=f"v{si}")
                for si in range(SC)]

        # broadcast bv across partitions via K=1 matmul
        with tc.tile_pool(name="ph0ps", bufs=1, space="PSUM") as ps0:
            bvb_ps = ps0.tile([128, D], F32, tag="bvb_ps")
            nc.tensor.matmul(bvb_ps[:], ones_sb[0:1, :], bv_row[0:1, :],
                             start=True, stop=True)
            bvb = pp.tile([128, D], F32, tag="bvb")
            nc.vector.tensor_copy(bvb[:], bvb_ps[:])

        # ---------------- phase 1: projections ----------------
        with tc.tile_pool(name="wpool", bufs=1) as wp, \
             tc.tile_pool(name="xpool", bufs=16) as xp, \
             tc.tile_pool(name="pj_ps", bufs=8, space="PSUM") as pjp:
            # x quarter 0 first so the first matmuls aren't queued behind 6MB
            # of weights on the HWDGE queue; then weights per-k-chunk.
            def load_x_quarter(q):
                t0 = q * 512
                xt = []
                for kk in range(KC):
                    xtile = xp.tile([128, 512], MD, tag="xt", name=f"xt{q}_{kk}")
                    nc.sync.dma_start(
                        xtile[:],
                        dma_cast(xT[kk * 128:(kk + 1) * 128, t0:t0 + 512]))
                    xt.append(xtile)
                return xt

            wq_sb = wp.tile([128, KC, D], MD, tag="wq_sb")
            wk_sb = wp.tile([128, KC, D], MD, tag="wk_sb")
            wv_sb = wp.tile([128, KC, D], MD, tag="wv_sb")
            # interleave x-quarter-0 with wq chunks so the first QT matmul's
            # operands land first on the single HWDGE queue
            xt0 = []
            for kk in range(KC):
                xtile = xp.tile([128, 512], MD, tag="xt", name=f"xt0_{kk}")
                nc.sync.dma_start(
                    xtile[:], dma_cast(xT[kk * 128:(kk + 1) * 128, 0:512]))
                xt0.append(xtile)
                nc.sync.dma_start(
                    wq_sb[:, kk, :], dma_cast(wq[kk * 128:(kk + 1) * 128, :]))
            for t_sb, t_dr in ((wk_sb, wk), (wv_sb, wv)):
                for kk in range(KC):
                    nc.sync.dma_start(
                        t_sb[:, kk, :],
                        dma_cast(t_dr[kk * 128:(kk + 1) * 128, :]))

            for q in range(4):        # t quarters of 512
                t0 = q * 512
                xt = xt0 if q == 0 else load_x_quarter(q)
                # QT[d, t] += wq[k, d].T @ xT[k, t]
                for j in range(4):
                    ps = pjp.tile([128, 512], F32, tag="pj", name=f"pq{q}_{j}")
                    for kk in range(KC):
                        nc.tensor.matmul(
                            ps[:], wq_sb[:, kk, j * 128:(j + 1) * 128],
                            xt[kk][:],
                            start=(kk == 0), stop=(kk == KC - 1))
                    nc.vector.tensor_scalar_add(
                        qt_sb[j][:, t0:t0 + 512], ps[:], bq_sb[:, j:j + 1])
                for j in range(4):
                    ps = pjp.tile([128, 512], F32, tag="pj", name=f"pk{q}_{j}")
                    for kk in range(KC):
                        nc.tensor.matmul(
                            ps[:], wk_sb[:, kk, j * 128:(j + 1) * 128],
                            xt[kk][:],
                            start=(kk == 0), stop=(kk == KC - 1))
                    nc.vector.tensor_scalar_add(
                        kt_sb[j][:, t0:t0 + 512], ps[:], bk_sb[:, j:j + 1])
                # V[s, d] += xT[k, s].T @ wv[k, d]
                for sl in range(4):
                    si = q * 4 + sl
                    ps = pjp.tile([128, 512], F32, tag="pj", name=f"pv{q}_{sl}")
                    for kk in range(KC):
                        nc.tensor.matmul(
                            ps[:], xt[kk][:, sl * 128:(sl + 1) * 128],
                            wv_sb[:, kk, :],
                            start=(kk == 0), stop=(kk == KC - 1))
                    vdst = v_sb[si].rearrange("p (h e) -> p h e", e=65)
                    nc.vector.tensor_add(
                        vdst[:, :, 0:64],
                        ps.rearrange("p (h e) -> p h e", e=64),
                        bvb.rearrange("p (h e) -> p h e", e=64))
                    nc.vector.tensor_copy(vdst[:, :, 64:65],
                                          ones_f[:, 0:8].unsqueeze(2))

        # ---------------- phases 2+3: attention + output proj ----------------
        with tc.tile_pool(name="attn", bufs=1) as ap_, \
             tc.tile_pool(name="exp_pool", bufs=4) as ep, \
             tc.tile_pool(name="sc_ps", bufs=2, space="PSUM") as scp, \
             tc.tile_pool(name="cx_ps", bufs=4, space="PSUM") as cxp, \
             tc.tile_pool(name="out_pool", bufs=2) as op_:

            wo_sb = ap_.tile([128, 4, C], MD, tag="wo_sb")
            nc.sync.dma_start(
                wo_sb[:], dma_cast(wo.ap().rearrange("(a p) c -> p a c", p=128)))
            ctx_sb = [ap_.tile([128, T], MD, tag=f"ctx{p}", name=f"ctx{p}")
                      for p in range(NP)]

            for g in range(NG):       # t halves of 1024
                gt = g * TG
                for p in range(NP):   # head pairs
                    hA, hB = 2 * p, 2 * p + 1
                    for tt in range(2):
                        q0 = gt + tt * 512
                        # ctx accumulators: [65, 512] per head
                        cxA = cxp.tile([128, 512], F32, tag="cx",
                                       name=f"cxA{g}{p}{tt}")
                        cxB = cxp.tile([128, 512], F32, tag="cx",
                                       name=f"cxB{g}{p}{tt}")
                        for si in range(SC):
                            s0 = si * 128
                            # scoresT psum: [A | B] cols of 512; the two
                            # K=64 matmuls land in row-groups 0-63/64-127
                            # and stream concurrently on the PE
                            sc = scp.tile([128, 1024], F32, tag="sc",
                                          name=f"sc{g}{p}{tt}{si}")
                            nc.tensor.matmul(
                                sc[:, 0:512],
                                kt_sb[p][0:64, s0:s0 + 128],
                                qt_sb[p][0:64, q0:q0 + 512],
                                start=True, stop=True)
                            nc.tensor.matmul(
                                sc[:, 512:1024],
                                kt_sb[p][64:128, s0:s0 + 128],
                                qt_sb[p][64:128, q0:q0 + 512],
                                start=True, stop=True)
                            et = ep.tile([128, 1024], MD, tag="et",
                                         name=f"et{g}{p}{tt}{si}")
                            nc.scalar.activation(et[:], sc[:], EXP, scale=scale)
                            # ctx[d(+denom), t] += Vaug[s, d].T @ expT[s, t]
                            nc.tensor.matmul(
                                cxA[0:65, :],
                                v_sb[si][:, hA * 65:hA * 65 + 65],
                                et[:, 0:512],
                                start=(si == 0), stop=(si == SC - 1))
                            nc.tensor.matmul(
                                cxB[0:65, :],
                                v_sb[si][:, hB * 65:hB * 65 + 65],
                                et[:, 512:1024],
                                start=(si == 0), stop=(si == SC - 1))
                        # normalize: ctxT[d, t] = ctx[d, t] * (1/denom[t])
                        for cxi, h in ((cxA, hA), (cxB, hB)):
                            rc = ep.tile([1, 512], F32, tag="rc", bufs=4,
                                         name=f"rc{g}{p}{tt}{h % 2}")
                            nc.vector.reciprocal(rc[0:1, :], cxi[64:65, :])
                            o = (h % 2) * 64
                            bc = ep.tile([128, 512], F32, tag="bcs", bufs=4,
                                         name=f"bcs{g}{p}{tt}{h % 2}")
                            nc.gpsimd.partition_broadcast(bc[:, :], rc[0:1, :],
                                                          channels=128)
                            dst = ctx_sb[p][o:o + 64, q0:q0 + 512]
                            nc.vector.tensor_copy(dst, cxi[0:64, :])
                            nc.vector.tensor_mul(dst, dst, bc[o:o + 64, :])

                # output projection for this t half
                for t8 in range(8):
                    tb = gt + t8 * 128
                    ob = op_.tile([128, C], F32, tag="ob", name=f"ob{g}{t8}")
                    for cc in range(2):
                        po = cxp.tile([128, 512], F32, tag="cx",
                                      name=f"po{g}{t8}{cc}")
                        for dd in range(4):
                            nc.tensor.matmul(
                                po[:], ctx_sb[dd][:, tb:tb + 128],
                                wo_sb[:, dd, cc * 512:(cc + 1) * 512],
                                start=(dd == 0), stop=(dd == 3))
                        nc.vector.tensor_copy(ob[:, cc * 512:(cc + 1) * 512],
                                              po[:])
                    nc.sync.dma_start(out[tb:tb + 128, :], ob[:])


def _shard_inputs(x, Wq, bq, Wk, bk, Wv, bv, Wo):
    in_maps = []
    for c in range(8):
        b, hg = c // 2, c % 2
        sl = slice(hg * D, (hg + 1) * D)
        in_maps.append({
            "xT": np.ascontiguousarray(np.asarray(x[b]).T),
            "wq": np.ascontiguousarray(np.asarray(Wq[:, sl])),
            "wk": np.ascontiguousarray(np.asarray(Wk[:, sl])),
            "wv": np.ascontiguousarray(np.asarray(Wv[:, sl])),
            "wo": np.ascontiguousarray(np.asarray(Wo[sl, :])),
            "bq": np.ascontiguousarray(np.asarray(bq[sl])),
            "bk": np.ascontiguousarray(np.asarray(bk[sl])),
            "bv": np.ascontiguousarray(np.asarray(bv[sl])),
        })
    return in_maps


class _Runner:
    """Persistent-jit SPMD runner (mirrors bass2jax.run_bass_via_pjrt but
    keeps the jitted callable alive so repeat kernel() calls don't
    re-trace). Our kernel writes every output element, so output buffers
    are passed non-donated."""

    def __init__(self, nc, n_cores=8):
        import jax
        from jax.sharding import Mesh, PartitionSpec
        from jax.experimental.shard_map import shard_map
        from concourse.bass2jax import (_bass_exec_p, install_neuronx_cc_hook,
                                        partition_id_tensor)
        install_neuronx_cc_hook()
        self.jax = jax
        self.n_cores = n_cores
        pid = nc.partition_id_tensor
        pid_name = pid.name if pid is not None else None

        in_names, out_names, out_avals, out_shapes = [], [], [], []
        for alloc in nc.m.functions[0].allocations:
            if not isinstance(alloc, mybir.MemoryLocationSet):
                continue
            name = alloc.memorylocations[0].name
            if alloc.kind == "ExternalInput":
                if name != pid_name:
                    in_names.append(name)
            elif alloc.kind == "ExternalOutput":
                out_names.append(name)
                shape = tuple(alloc.tensor_shape)
                out_shapes.append(shape)
                out_avals.append(
                    jax.core.ShapedArray(shape, mybir.dt.np(alloc.dtype)))
        self.in_names = in_names
        self.out_names = out_names
        self.out_shapes = out_shapes
        n_params = len(in_names)
        all_in_names = list(in_names) + list(out_names)
        if pid_name is not None:
            all_in_names.append(pid_name)

        def _body(*args):
            operands = list(args)
            if pid_name is not None:
                operands.append(partition_id_tensor())
            return tuple(_bass_exec_p.bind(
                *operands,
                out_avals=tuple(out_avals),
                in_names=tuple(all_in_names),
                out_names=tuple(out_names),
                lowering_input_output_aliases=(),
                sim_require_finite=True,
                sim_require_nnan=True,
                nc=nc,
            ))

        devices = jax.devices()[:n_cores]
        mesh = Mesh(np.asarray(devices), ("core",))
        n_outs = len(out_names)
        self.fn = jax.jit(shard_map(
            _body, mesh=mesh,
            in_specs=(PartitionSpec("core"),) * (n_params + n_outs),
            out_specs=(PartitionSpec("core"),) * n_outs,
            check_rep=False), keep_unused=True)
        self.zero_outs = [
            np.zeros((n_cores * s[0], *s[1:]), np.float32) for s in out_shapes]

    def run(self, in_maps):
        args = [np.concatenate([np.asarray(m[name]) for m in in_maps], axis=0)
                for name in self.in_names]
        out_arrs = self.fn(*args, *self.zero_outs)
        self.jax.block_until_ready(out_arrs)
        per_core = []
        for c in range(self.n_cores):
            d = {}
            for i, name in enumerate(self.out_names):
                shape = self.out_shapes[i]
                d[name] = np.asarray(out_arrs[i]).reshape(
                    self.n_cores, *shape)[c]
            per_core.append(d)
        return per_core


def kernel(x, Wq, bq, Wk, bk, Wv, bv, Wo, bo):
    """Full inputs in, full [4, 2048, 1024] float32 output out."""
    if "runner" not in _CACHE:
        _CACHE["runner"] = _Runner(_build_program())
    in_maps = _shard_inputs(x, Wq, bq, Wk, bk, Wv, bv, Wo)
    results = _CACHE["runner"].run(in_maps)

    bo = np.asarray(bo, dtype=np.float32)
    out = np.empty((4, T, C), np.float32)
    for b in range(4):
        out[b] = results[2 * b]["out"] + results[2 * b + 1]["out"] + bo
    return out
